# revision 21
# baseline (speedup 1.0000x reference)
"""Trainium2 Bass kernel for nn_DecoderAttention (Bahdanau decoder step).

Sharding (8 cores):
  - attention + LSTM: data-parallel over batch (8 batches per core)
  - fc vocab projection: vocab-sharded (6283 cols/core, padded to 6656),
    with an on-device AllGather of h_new^T between the phases.

Attention works in a transposed orientation: per 512-wide s-block,
proj^T[o, s] accumulates in PSUM, the per-batch proj_s row + bias enters as
the ACT per-partition bias during tanh, and scores come from accumulating
PE matmuls of e^T against va. PE matmuls run in bf16 (fp32 accumulation);
softmax bookkeeping and LSTM pointwise math stay fp32.
"""

import ml_dtypes
import numpy as np

import concourse.bass as bass
import concourse.mybir as mybir
import concourse.tile as tile
from concourse import bacc, masks
from concourse.bass import ts
from concourse.bass_isa import ReduceOp
from concourse.bass_utils import run_bass_kernel_spmd

# problem constants (hardcoded per contract)
V, E, H = 50257, 512, 1024
B, S = 64, 2048
NCORES = 8
BL = B // NCORES          # local batches per core = 8
SC = S // 128             # 128-row s-chunks per batch = 16
NB = S // 512             # 512-row s-blocks per batch = 4
HC = H // 128             # h/o-chunks = 8
EC = E // 128             # emb chunks = 4
KC = EC + HC + HC         # lstm contraction chunks = 20
VSH = (V + NCORES - 1) // NCORES       # 6283 true vocab cols per core
VP = 6656                 # padded vocab cols per core (13 * 512)
NV = VP // 512            # 13 fc column tiles

F32 = mybir.dt.float32
BF16 = mybir.dt.bfloat16
NPBF = ml_dtypes.bfloat16

AF = mybir.ActivationFunctionType
ALU = mybir.AluOpType
AX = mybir.AxisListType


def build_nc(phase="full", nbatch=BL):
    nc = bacc.Bacc("TRN2", target_bir_lowering=False, num_devices=NCORES)

    # ---- dram I/O (per-core shapes); bf16 for PE-side operands ----
    enc = nc.dram_tensor("enc", [BL, S, H], BF16, kind="ExternalInput")
    uaT = nc.dram_tensor("uaT", [H, H], BF16, kind="ExternalInput")
    waT = nc.dram_tensor("waT", [H, H], BF16, kind="ExternalInput")
    abT_r = nc.dram_tensor("abT_r", [128, HC], F32, kind="ExternalInput")
    vaT_r = nc.dram_tensor("vaT_r", [128, HC], BF16, kind="ExternalInput")
    stT_r = nc.dram_tensor("stT_r", [128, HC * BL], BF16, kind="ExternalInput")
    embT_r = nc.dram_tensor("embT_r", [128, EC * BL], BF16, kind="ExternalInput")
    cell_sl = nc.dram_tensor("cell_sl", [B, 128], F32, kind="ExternalInput")
    wcat_sl = nc.dram_tensor("wcat_sl", [KC * 128, 512], BF16, kind="ExternalInput")
    bias_sl = nc.dram_tensor("bias_sl", [1, 512], BF16, kind="ExternalInput")
    fcT = nc.dram_tensor("fcT", [H, VP], BF16, kind="ExternalInput")
    fcb = nc.dram_tensor("fcb", [1, VP], BF16, kind="ExternalInput")

    pred = nc.dram_tensor("pred", [B, VP], F32, kind="ExternalOutput")
    h_out = nc.dram_tensor("h_out", [B, 128], F32, kind="ExternalOutput")
    c_out = nc.dram_tensor("c_out", [B, 128], F32, kind="ExternalOutput")

    with tile.TileContext(nc) as tc:
        from contextlib import ExitStack

        with ExitStack() as ctx0:
            # ---------- persistent pools ----------
            const_pool = ctx0.enter_context(tc.tile_pool(name="const", bufs=1))
            ident = const_pool.tile([128, 128], BF16)
            masks.make_identity(nc, ident[:])
            ones_bf = const_pool.tile([1, 128], BF16)
            nc.gpsimd.memset(ones_bf[:], 1.0)

            # lstm_in^T layout [128, kc*8+b]: cols 0:32 emb, 32:96 ctx, 96:160 s_t
            lstm_pool = ctx0.enter_context(tc.tile_pool(name="lstm_in", bufs=1))
            lstm_in = lstm_pool.tile([128, KC * BL], BF16)
            nc.gpsimd.dma_start(lstm_in[:, 0 : EC * BL], embT_r[:, :])
            nc.gpsimd.dma_start(lstm_in[:, (EC + HC) * BL : KC * BL], stT_r[:, :])

            hall_pool = ctx0.enter_context(tc.tile_pool(name="hall", bufs=1))
            hallT = hall_pool.tile([128, NCORES * 64], BF16)  # [p, hc*64+r*8+b]

            # ================= PHASE A: attention =================
            with ExitStack() as ctxA:
                rowp = ctxA.enter_context(tc.tile_pool(name="rowp", bufs=1))
                abT_sb = rowp.tile([128, HC], F32)
                nc.gpsimd.dma_start(abT_sb[:], abT_r[:, :])
                vaT_sb = rowp.tile([128, HC], BF16)
                nc.gpsimd.dma_start(vaT_sb[:], vaT_r[:, :])
                r8T = rowp.tile([128, HC * BL], F32)  # [p, oc*8+b]

                ua_pool = ctxA.enter_context(tc.tile_pool(name="ua", bufs=1))
                wa_pool = ctxA.enter_context(tc.tile_pool(name="wa", bufs=2))

                # Ua_w^T resident in SBUF: [128, hc*1024 + o]
                ua_sb = ua_pool.tile([128, HC * H], BF16)
                nc.sync.dma_start(
                    ua_sb[:].rearrange("p (hc o) -> p hc o", o=H),
                    uaT[:, :].rearrange("(hc p) o -> p hc o", p=128),
                )

                # proj_s^T: r8T[o, b] = (Wa_w @ s_t^T)[o, b] + (Wa_b + Ua_b)[o]
                with tc.tile_pool(name="ps_ps", bufs=8, space="PSUM") as ps_ps:
                    psT = [
                        ps_ps.tile([128, BL], F32, tag="ps", name=f"psT{oc}")
                        for oc in range(HC)
                    ]
                    for hc in range(HC):
                        wa_t = wa_pool.tile([128, H], BF16, name="wa_t")
                        nc.sync.dma_start(wa_t[:], waT[ts(hc, 128), :])
                        for oc in range(HC):
                            nc.tensor.matmul(
                                psT[oc][:],
                                wa_t[:, ts(oc, 128)],
                                lstm_in[:, (EC + HC + hc) * BL : (EC + HC + hc + 1) * BL],
                                start=(hc == 0),
                                stop=(hc == HC - 1),
                            )
                    for oc in range(HC):
                        nc.scalar.activation(
                            r8T[:, ts(oc, BL)],
                            psT[oc][:],
                            AF.Identity,
                            bias=abT_sb[:, oc : oc + 1],
                        )

                tr_ps = ctxA.enter_context(
                    tc.tile_pool(name="tr_ps", bufs=2, space="PSUM")
                )
                pj_ps = ctxA.enter_context(
                    tc.tile_pool(name="pj_ps", bufs=4, space="PSUM")
                )
                sm_ps = ctxA.enter_context(
                    tc.tile_pool(name="sm_ps", bufs=2, space="PSUM")
                )
                enc_pool = ctxA.enter_context(tc.tile_pool(name="encp", bufs=NB + 1))
                encT_pool = ctxA.enter_context(tc.tile_pool(name="encT", bufs=2))
                e_pool = ctxA.enter_context(tc.tile_pool(name="eT", bufs=9))
                at_pool = ctxA.enter_context(tc.tile_pool(name="attn", bufs=2))
                sm_sb = ctxA.enter_context(tc.tile_pool(name="sm_sb", bufs=3))

                # ---- per-batch attention ----
                for b in range(nbatch):
                    attn = at_pool.tile([128, SC], BF16, name="attn")
                    enc_blks = []
                    for blk in range(NB):
                        # one 1MB DMA per 512-row block: [p, ssub, h]
                        eb = enc_pool.tile([128, 4 * H], BF16, tag="enc", name="eb")
                        enc_blks.append(eb)
                        nc.sync.dma_start(
                            eb[:].rearrange("p (u h) -> p u h", h=H),
                            enc[b, blk * 512 : (blk + 1) * 512, :].rearrange(
                                "(u p) h -> p u h", p=128
                            ),
                        )

                        # transpose to encT[h%128, hc*512 + s'] (bf16)
                        encT = encT_pool.tile([128, HC * 512], BF16, name="encT")
                        for hp in range(4):  # hc pairs
                            trp = tr_ps.tile([128, 1024], BF16, tag="tr", name="trp")
                            for hci in range(2):
                                hc = 2 * hp + hci
                                for ssub in range(4):
                                    nc.tensor.transpose(
                                        trp[:, (hci * 4 + ssub) * 128 : (hci * 4 + ssub + 1) * 128],
                                        eb[:, ssub * H + hc * 128 : ssub * H + (hc + 1) * 128],
                                        ident[:],
                                    )
                            nc.scalar.copy(encT[:, hp * 1024 : (hp + 1) * 1024], trp[:])

                        # proj^T + tanh(..+r) + scores, two oc passes of 4
                        scp = sm_ps.tile([128, 4], F32, tag="sm", name="scp")
                        eTs = []
                        for opass in range(2):
                            for oci in range(4):
                                oc = opass * 4 + oci
                                pj = pj_ps.tile([128, 512], F32, tag="pj", name="pj")
                                for hc in range(HC):
                                    nc.tensor.matmul(
                                        pj[:],
                                        ua_sb[:, hc * H + oc * 128 : hc * H + (oc + 1) * 128],
                                        encT[:, ts(hc, 512)],
                                        start=(hc == 0),
                                        stop=(hc == HC - 1),
                                    )
                                eT = e_pool.tile([128, 512], BF16, tag="eT", name="eT")
                                eTs.append(eT)
                                nc.scalar.activation(
                                    eT[:],
                                    pj[:],
                                    AF.Tanh,
                                    bias=r8T[:, oc * BL + b : oc * BL + b + 1],
                                )
                        for ssub in range(4):
                            for oc in range(HC):
                                nc.tensor.matmul(
                                    scp[:, ssub : ssub + 1],
                                    eTs[oc][:, ts(ssub, 128)],
                                    vaT_sb[:, oc : oc + 1],
                                    start=(oc == 0),
                                    stop=(oc == HC - 1),
                                )
                        nc.scalar.activation(
                            attn[:, blk * 4 : (blk + 1) * 4], scp[:], AF.Exp
                        )

                    # softmax denominator (scores bounded; no max-sub needed)
                    zcol = sm_sb.tile([128, 1], F32, tag="zc", name="zcol")
                    nc.vector.reduce_sum(zcol[:], attn[:], axis=AX.X)
                    zsum = sm_sb.tile([128, 1], F32, tag="zs", name="zsum")
                    nc.gpsimd.partition_all_reduce(zsum[:], zcol[:], 128, ReduceOp.add)
                    zr = sm_sb.tile([128, 1], F32, tag="zr", name="zr")
                    nc.vector.reciprocal(zr[:], zsum[:])

                    # context accumulation over s (per h-chunk)
                    ctxp = sm_ps.tile([128, HC], F32, tag="sm", name="ctxp")
                    for hc in range(HC):
                        for t in range(SC):
                            blk, ssub = divmod(t, 4)
                            nc.tensor.matmul(
                                ctxp[:, hc : hc + 1],
                                enc_blks[blk][:, ssub * H + hc * 128 : ssub * H + (hc + 1) * 128],
                                attn[:, t : t + 1],
                                start=(t == 0),
                                stop=(t == SC - 1),
                            )
                    # scale by 1/Z into lstm_in ctx cols (32 + hc*8 + b)
                    dst = lstm_in[:].rearrange("p (k e) -> p k e", e=BL)[
                        :, EC : EC + HC, b
                    ]
                    nc.vector.tensor_scalar_mul(dst, ctxp[:, 0:HC], zr[:])

            # ================= PHASE B: LSTM (h-sliced: this core computes
            # gate/hidden columns c*128..c*128+128 for ALL 64 batches) ======
            with ExitStack() as ctxB:
              if phase != "A":
                wcs_pool = ctxB.enter_context(tc.tile_pool(name="wcs", bufs=1))
                gsb_pool = ctxB.enter_context(tc.tile_pool(name="gsb", bufs=1))
                pw_pool = ctxB.enter_context(tc.tile_pool(name="pw", bufs=1))
                lall_pool = ctxB.enter_context(tc.tile_pool(name="lall", bufs=1))
                dram_pool = ctxB.enter_context(
                    tc.tile_pool(name="dram", bufs=1, space="DRAM")
                )

                # gather lstm_in^T from all cores -> [128, kc*64 + r*8 + b]
                li_d = dram_pool.tile([128, KC * BL], BF16, name="li_d")
                lall_d = dram_pool.tile([NCORES * 128, KC * BL], BF16, name="lall_d")
                nc.gpsimd.dma_start(li_d[:], lstm_in[:])
                nc.gpsimd.collective_compute(
                    "AllGather",
                    ALU.bypass,
                    replica_groups=[list(range(NCORES))],
                    ins=[li_d[:].opt()],
                    outs=[lall_d[:].opt()],
                )
                # l_all cols: r*(KC*BL) + kc*BL + b  (rank-major, contiguous DMA)
                l_all = lall_pool.tile([128, KC * B], BF16)
                nc.gpsimd.dma_start(
                    l_all[:].rearrange("p (r q) -> p r q", q=KC * BL),
                    lall_d[:].rearrange("(r p) q -> p r q", p=128),
                )
                l_kc = lall_pool.tile([128, KC * B], BF16, name="l_kc")
                nc.vector.tensor_copy(
                    l_kc[:].rearrange("p (k r e) -> p k r e", r=NCORES, e=BL),
                    l_all[:].rearrange("p (r k e) -> p k r e", r=NCORES, e=BL),
                )

                # this core's [Wih|Whh]^T gate-column slice (i|f|g|o x 128)
                wcs = wcs_pool.tile([128, KC * 512], BF16)
                nc.sync.dma_start(
                    wcs[:].rearrange("p (k g) -> p k g", g=512),
                    wcat_sl[:, :].rearrange("(k p) g -> p k g", p=128),
                )

                gates = gsb_pool.tile([B, 512], F32)
                with tc.tile_pool(name="g_ps", bufs=1, space="PSUM") as g_ps:
                    gp = g_ps.tile([B, 512], F32, name="gp")
                    for kc in range(KC):
                        nc.tensor.matmul(
                            gp[:],
                            l_kc[:, ts(kc, B)],
                            wcs[:, ts(kc, 512)],
                            start=(kc == 0),
                            stop=False,
                        )
                    brow = pw_pool.tile([1, 512], BF16, tag="br")
                    nc.gpsimd.dma_start(brow[:], bias_sl[:, :])
                    nc.tensor.matmul(
                        gp[:],
                        ones_bf[0:1, 0:B],
                        brow[0:1, :],
                        start=False,
                        stop=True,
                    )
                    nc.scalar.copy(gates[:], gp[:])

                i_s = pw_pool.tile([B, 128], F32)
                f_s = pw_pool.tile([B, 128], F32)
                o_s = pw_pool.tile([B, 128], F32)
                g_t = pw_pool.tile([B, 128], F32)
                nc.scalar.activation(i_s[:], gates[:, 0:128], AF.Sigmoid)
                nc.scalar.activation(f_s[:], gates[:, 128:256], AF.Sigmoid)
                nc.scalar.activation(o_s[:], gates[:, 384:512], AF.Sigmoid)
                nc.scalar.activation(g_t[:], gates[:, 256:384], AF.Tanh)

                cell_sb = pw_pool.tile([B, 128], F32, tag="c1")
                nc.gpsimd.dma_start(cell_sb[:], cell_sl[:, :])
                t1 = pw_pool.tile([B, 128], F32, tag="c2")
                nc.vector.tensor_mul(t1[:], f_s[:], cell_sb[:])
                t2 = pw_pool.tile([B, 128], F32, tag="c3")
                nc.vector.tensor_mul(t2[:], i_s[:], g_t[:])
                c_new = pw_pool.tile([B, 128], F32, tag="c4")
                nc.vector.tensor_add(c_new[:], t1[:], t2[:])
                nc.gpsimd.dma_start(c_out[:, :], c_new[:])

                tc_t = pw_pool.tile([B, 128], F32, tag="c5")
                nc.scalar.activation(tc_t[:], c_new[:], AF.Tanh)
                h_new = pw_pool.tile([B, 128], F32, tag="c6")
                nc.vector.tensor_mul(h_new[:], o_s[:], tc_t[:])
                nc.gpsimd.dma_start(h_out[:, :], h_new[:])

                # h_new^T for fc: [128 (h in this slice), 64 b]
                hbf = pw_pool.tile([B, 128], BF16, tag="c7")
                nc.vector.tensor_copy(hbf[:], h_new[:])
                with tc.tile_pool(name="c_ps", bufs=1, space="PSUM") as c_ps:
                    hTp = c_ps.tile([128, B], BF16)
                    nc.tensor.transpose(hTp[:], hbf[:], ident[0:B, 0:B])
                    hT_sb = pw_pool.tile([128, B], BF16, tag="c8")
                    nc.scalar.copy(hT_sb[:], hTp[:])

                # AllGather h^T slices -> hallT[p, hc*64 + b] (hc == rank)
                hT_d = dram_pool.tile([128, B], BF16, name="hT_d")
                hall_d = dram_pool.tile([NCORES * 128, B], BF16, name="hall_d")
                nc.gpsimd.dma_start(hT_d[:], hT_sb[:])
                nc.gpsimd.collective_compute(
                    "AllGather",
                    ALU.bypass,
                    replica_groups=[list(range(NCORES))],
                    ins=[hT_d[:].opt()],
                    outs=[hall_d[:].opt()],
                )
                nc.gpsimd.dma_start(
                    hallT[:].rearrange("p (r e) -> p r e", e=B),
                    hall_d[:].rearrange("(r p) e -> p r e", p=128),
                )

            # ================= PHASE C: fc =================
            with ExitStack() as ctxC:
              if phase == "full":
                f_ps = ctxC.enter_context(
                    tc.tile_pool(name="f_ps", bufs=2, space="PSUM")
                )
                fcv_pool = ctxC.enter_context(tc.tile_pool(name="fcv", bufs=4))
                pr_pool = ctxC.enter_context(tc.tile_pool(name="pr", bufs=2))
                fb_pool = ctxC.enter_context(tc.tile_pool(name="fb", bufs=2))

                for vc in range(NV):
                    fv = fcv_pool.tile([128, HC * 512], BF16, name="fv")
                    nc.sync.dma_start(
                        fv[:].rearrange("p (hc v) -> p hc v", v=512),
                        fcT[:, ts(vc, 512)].rearrange("(hc p) v -> p hc v", p=128),
                    )
                    fp = f_ps.tile([B, 512], F32, name="fp")
                    for hc in range(HC):
                        nc.tensor.matmul(
                            fp[:],
                            hallT[:, ts(hc, 64)],
                            fv[:, ts(hc, 512)],
                            start=(hc == 0),
                            stop=False,
                        )
                    fbr = fb_pool.tile([1, 512], BF16, name="fbr")
                    nc.gpsimd.dma_start(fbr[:], fcb[:, ts(vc, 512)])
                    nc.tensor.matmul(
                        fp[:],
                        ones_bf[0:1, 0:B],
                        fbr[0:1, :],
                        start=False,
                        stop=True,
                    )
                    prt = pr_pool.tile([B, 512], F32, name="prt")
                    nc.scalar.copy(prt[:], fp[:])
                    nc.gpsimd.dma_start(pred[:, ts(vc, 512)], prt[:])

    nc.compile()
    return nc


_NC_CACHE = None


def _get_nc():
    global _NC_CACHE
    if _NC_CACHE is None:
        _NC_CACHE = build_nc()
    return _NC_CACHE


def kernel(x, hidden, cell, encoder_outputs, emb, Wa_w, Wa_b, Ua_w, Ua_b,
           va_w, va_b, Wih, Whh, bih, bhh, fc_w, fc_b, **run_kwargs):
    x = np.asarray(x)
    hidden = np.asarray(hidden, np.float32)
    cell = np.asarray(cell, np.float32)
    encoder_outputs = np.asarray(encoder_outputs, np.float32)
    emb = np.asarray(emb, np.float32)
    Wa_w = np.asarray(Wa_w, np.float32)
    Wa_b = np.asarray(Wa_b, np.float32)
    Ua_w = np.asarray(Ua_w, np.float32)
    Ua_b = np.asarray(Ua_b, np.float32)
    va_w = np.asarray(va_w, np.float32)
    va_b = np.asarray(va_b, np.float32)
    Wih = np.asarray(Wih, np.float32)
    Whh = np.asarray(Whh, np.float32)
    bih = np.asarray(bih, np.float32)
    bhh = np.asarray(bhh, np.float32)
    fc_w = np.asarray(fc_w, np.float32)
    fc_b = np.asarray(fc_b, np.float32)

    # ---- host-side layout prep (weights + tiny tensors) ----
    enc_bf = encoder_outputs.astype(NPBF)                    # [B, S, H]
    uaT = np.ascontiguousarray(Ua_w.T.astype(NPBF))          # [H, H]
    waT = np.ascontiguousarray(Wa_w.T.astype(NPBF))          # [H, H]
    ab = (Wa_b + Ua_b).astype(np.float32)
    abT_r = np.ascontiguousarray(ab.reshape(HC, 128).T)      # [128, oc]
    vaT_r = np.ascontiguousarray(va_w.reshape(HC, 128).T.astype(NPBF))

    s_t = hidden[-1]                                         # [B, H]
    embedded = emb[x[:, 0]]                                  # [B, E]

    # Wcat^T rows: [Wih emb cols | Wih ctx cols | Whh] -> [KC*128, 4H];
    # per-core gate-column slices (i|f|g|o x 128 for h-slice c)
    wcatT_full = np.concatenate([Wih[:, :E].T, Wih[:, E:].T, Whh.T], axis=0)
    bias_full = bih + bhh

    fcT_pad = np.zeros((H, NCORES * VP), NPBF)
    fcb_pad = np.zeros((1, NCORES * VP), NPBF)
    fc_wT = fc_w.T.astype(NPBF)                              # [H, V]
    for c in range(NCORES):
        lo = c * VSH
        n = min(VSH, V - lo)
        fcT_pad[:, c * VP : c * VP + n] = fc_wT[:, lo : lo + n]
        fcb_pad[0, c * VP : c * VP + n] = fc_b[lo : lo + n].astype(NPBF)

    def pack_T(a):  # [BL, D] f32 -> bf16 [128, (D//128)*BL] cols kc*BL+b
        d = a.shape[1]
        return np.ascontiguousarray(
            a.T.reshape(d // 128, 128, BL).transpose(1, 0, 2).reshape(128, -1)
        ).astype(NPBF)

    in_maps = []
    for c in range(NCORES):
        bs = slice(c * BL, (c + 1) * BL)
        gidx = np.concatenate(
            [g * H + np.arange(c * 128, (c + 1) * 128) for g in range(4)]
        )
        in_maps.append({
            "enc": np.ascontiguousarray(enc_bf[bs]),
            "uaT": uaT,
            "waT": waT,
            "abT_r": abT_r,
            "vaT_r": vaT_r,
            "stT_r": pack_T(s_t[bs]),
            "embT_r": pack_T(embedded[bs]),
            "cell_sl": np.ascontiguousarray(cell[-1][:, c * 128 : (c + 1) * 128]),
            "wcat_sl": np.ascontiguousarray(wcatT_full[:, gidx].astype(NPBF)),
            "bias_sl": np.ascontiguousarray(
                bias_full[gidx].reshape(1, 512).astype(NPBF)
            ),
            "fcT": np.ascontiguousarray(fcT_pad[:, c * VP : (c + 1) * VP]),
            "fcb": np.ascontiguousarray(fcb_pad[:, c * VP : (c + 1) * VP]),
        })

    nc = _get_nc()
    res = run_bass_kernel_spmd(nc, in_maps, core_ids=list(range(NCORES)), **run_kwargs)
    results = res.results

    predictions = np.zeros((B, V), np.float32)
    h_new = np.zeros((B, H), np.float32)
    c_new = np.zeros((B, H), np.float32)
    for c in range(NCORES):
        lo = c * VSH
        n = min(VSH, V - lo)
        predictions[:, lo : lo + n] = results[c]["pred"][:, :n]
        h_new[:, c * 128 : (c + 1) * 128] = results[c]["h_out"]
        c_new[:, c * 128 : (c + 1) * 128] = results[c]["c_out"]

    if run_kwargs:
        kernel.last_results = res
    return predictions[:, None, :], h_new[None], c_new[None]
